# revision 5
# baseline (speedup 1.0000x reference)
"""Trainium2 multi-head attention kernel (8 NeuronCores, data-parallel over batch).

Reference computation (per batch element b):
    qw = q @ Wq, kw = k @ Wk, vw = v @ Wv   (each reshaped to [S, H=8, Dh=64])
    a  = softmax(qw kw^T / sqrt(Dh) - (1-v_mask)*1e10, axis=k)
    o  = (a @ vw) * q_mask

Per-core layout strategy (core i handles batch element i):
  - Host passes q^T/k^T/v^T ([D=512, S=2048], bf16) so the contraction dim (d)
    lands on SBUF partitions for the projection matmuls.
  - Projections produce qw^T/kw^T ([512, 2048]; head h on partitions
    (h%2)*64..+64 of tile h//2) and vw in natural layout, padded per head with
    a denominator-ones column and pre-scaled by v_mask ([128, 16*520]).
  - Scores are computed transposed ([sk, sq]) two heads at a time via PE
    row-tiling (K=64 at array rows 0-63 / 64-127), exp'd on ScalarE
    (scale=1/8 folded into the activation) into bf16.
  - AV uses exp^T as the stationary operand: out[sq,65] accumulates over sk
    in PSUM; col 64 is the softmax denominator (ones column of vw_pad).
  - Epilogue: out = psum[:, :64] * reciprocal(psum[:, 64]) * q_mask, DMA'd
    straight to DRAM in natural [S, 512] layout.

softmax is computed without max subtraction: logits are ~N(0,1) for these
input distributions (|logit| < ~7), so exp() cannot overflow, and masked
keys are handled by zeroing vw_pad rows (identical to the reference's
additive -1e10 mask).
"""

import numpy as np

B, S, D = 8, 2048, 512
H, DH = 8, 64
P = 128            # partitions
NDC = D // P       # 4 d-chunks
NT = S // P        # 16 sk tiles
NJ = 4             # sq chunks of 512
NU = 4             # sq subtiles of 128 per chunk
VW_STRIDE = 520    # 8 heads * 65 (64 vw cols + 1 ones col)
N_CORES = 8

_cached = {}


def _build():
    import concourse.bacc as bacc
    import concourse.mybir as mybir
    import concourse.tile as tile

    f32 = mybir.dt.float32
    bf16 = mybir.dt.bfloat16

    nc = bacc.Bacc("TRN2", target_bir_lowering=False, debug=False,
                   num_devices=N_CORES)

    qT = nc.dram_tensor("qT", [D, S], bf16, kind="ExternalInput")
    kT = nc.dram_tensor("kT", [D, S], bf16, kind="ExternalInput")
    vT = nc.dram_tensor("vT", [D, S], bf16, kind="ExternalInput")
    Wq = nc.dram_tensor("Wq", [D, D], bf16, kind="ExternalInput")
    Wk = nc.dram_tensor("Wk", [D, D], bf16, kind="ExternalInput")
    Wv = nc.dram_tensor("Wv", [D, D], bf16, kind="ExternalInput")
    vm = nc.dram_tensor("vm", [P, NT], f32, kind="ExternalInput")
    qm = nc.dram_tensor("qm", [P, NT], f32, kind="ExternalInput")
    out = nc.dram_tensor("out", [S, D], f32, kind="ExternalOutput")

    with tile.TileContext(nc) as tc:
        with (
            tc.tile_pool(name="persist", bufs=1) as pp,
            tc.tile_pool(name="exppool", bufs=2) as ep,
            tc.tile_pool(name="outstage", bufs=8) as op,
            tc.tile_pool(name="small", bufs=8) as sp,
        ):
            # persistent activations
            qwT = [pp.tile([P, S], bf16, name=f"qwT{i}", tag=f"qwT{i}") for i in range(NDC)]
            kwT = [pp.tile([P, S], bf16, name=f"kwT{i}", tag=f"kwT{i}") for i in range(NDC)]
            vw_pad = pp.tile([P, NT * VW_STRIDE], bf16, name="vw_pad", tag="vw_pad")
            vm_sb = pp.tile([P, NT], f32, name="vm_sb", tag="vm")
            qm_sb = pp.tile([P, NT], f32, name="qm_sb", tag="qm")
            ones8 = pp.tile([P, H], f32, name="ones8", tag="ones8")

            # ---------------- Phase A: projections ----------------
            with (
                tc.tile_pool(name="inpool", bufs=1) as ip,
                tc.tile_pool(name="ps_proj", bufs=4, space="PSUM") as ps_p,
            ):
                nc.sync.dma_start(vm_sb[:], vm.ap())
                nc.sync.dma_start(qm_sb[:], qm.ap())
                nc.gpsimd.memset(ones8[:], 1.0)

                qT_sb = [ip.tile([P, S], bf16, name=f"qT{i}", tag=f"qT{i}") for i in range(NDC)]
                kT_sb = [ip.tile([P, S], bf16, name=f"kT{i}", tag=f"kT{i}") for i in range(NDC)]
                vT_sb = [ip.tile([P, S], bf16, name=f"vT{i}", tag=f"vT{i}") for i in range(NDC)]
                Wq_sb = [ip.tile([P, D], bf16, name=f"Wq{i}", tag=f"Wq{i}") for i in range(NDC)]
                Wk_sb = [ip.tile([P, D], bf16, name=f"Wk{i}", tag=f"Wk{i}") for i in range(NDC)]
                Wv_sb = [ip.tile([P, D], bf16, name=f"Wv{i}", tag=f"Wv{i}") for i in range(NDC)]
                for dc in range(NDC):
                    sl = slice(dc * P, (dc + 1) * P)
                    nc.sync.dma_start(qT_sb[dc][:], qT.ap()[sl, :])
                    nc.sync.dma_start(kT_sb[dc][:], kT.ap()[sl, :])
                    nc.sync.dma_start(vT_sb[dc][:], vT.ap()[sl, :])
                    nc.sync.dma_start(Wq_sb[dc][:], Wq.ap()[sl, :])
                    nc.sync.dma_start(Wk_sb[dc][:], Wk.ap()[sl, :])
                    nc.sync.dma_start(Wv_sb[dc][:], Wv.ap()[sl, :])

                # qwT / kwT: [o, s] = Wx[d, o].T @ xT[d, s]
                for W_sb, xT_sb, dstT in ((Wq_sb, qT_sb, qwT),
                                          (Wk_sb, kT_sb, kwT)):
                    for i in range(NDC):  # o tile
                        ps = [ps_p.tile([P, 512], f32, name="ps_proj", tag="ps_proj")
                              for _ in range(NJ)]
                        for dc in range(NDC):
                            for n in range(NJ):
                                nc.tensor.matmul(
                                    ps[n][:],
                                    W_sb[dc][:, i * P:(i + 1) * P],
                                    xT_sb[dc][:, n * 512:(n + 1) * 512],
                                    start=(dc == 0), stop=(dc == NDC - 1),
                                )
                        for n in range(NJ):
                            nc.vector.tensor_copy(
                                dstT[i][:, n * 512:(n + 1) * 512], ps[n][:])

                # vw (natural [s, o]) into padded per-head layout, v_mask folded
                for m in range(NT):
                    ps = ps_p.tile([P, 512], f32, name="ps_projv", tag="ps_proj")
                    for dc in range(NDC):
                        nc.tensor.matmul(
                            ps[:],
                            vT_sb[dc][:, m * P:(m + 1) * P],
                            Wv_sb[dc][:],
                            start=(dc == 0), stop=(dc == NDC - 1),
                        )
                    base = m * VW_STRIDE
                    dst = vw_pad[:, base:base + VW_STRIDE].rearrange(
                        "p (h c) -> p h c", c=65)[:, :, 0:64]
                    src = ps.rearrange("p (h c) -> p h c", c=64)
                    nc.vector.tensor_scalar_mul(dst, src, vm_sb[:, m:m + 1])
                    ones_slots = vw_pad[:, base:base + VW_STRIDE].rearrange(
                        "p (h c) -> p h c", c=65)[:, :, 64:65]
                    nc.vector.tensor_scalar_mul(
                        ones_slots, ones8.rearrange("p (h c) -> p h c", c=1),
                        vm_sb[:, m:m + 1])

            # ---------------- Phase B: attention ----------------
            ps_s_cm = tc.tile_pool(name="ps_scores", bufs=3, space="PSUM")
            ps_av_cm = tc.tile_pool(name="ps_av", bufs=2, space="PSUM")
            ps_s = ps_s_cm.__enter__()
            ps_av = ps_av_cm.__enter__()
            Exp = mybir.ActivationFunctionType.Exp

            def emit_scores_exp(pair, j, g, exp_sb):
                """Scores + exp for sk tiles (2g, 2g+1), both heads of pair."""
                for h in range(2):
                    prt = slice(h * 64, (h + 1) * 64)
                    ps = ps_s.tile([P, 1024], f32, name="ps_scores", tag="scores")
                    for tt in range(2):
                        t = 2 * g + tt
                        nc.tensor.matmul(
                            ps[:, tt * 512:(tt + 1) * 512],
                            kwT[pair][prt, t * P:(t + 1) * P],
                            qwT[pair][prt, j * 512:(j + 1) * 512],
                            start=True, stop=True,
                        )
                    nc.scalar.activation(
                        exp_sb[h][:, (2 * g) * 512:(2 * g + 2) * 512],
                        ps[:], Exp, scale=0.125)

            def emit_av_group(pair, j, u, h, exp_sb, out_t):
                """AV accumulation + epilogue for (sq subtile u, head h)."""
                gh = pair * 2 + h  # global head for vw_pad column block
                ps_o = ps_av.tile([P, 65], f32, name="ps_o", tag="av")
                for t in range(NT):
                    nc.tensor.matmul(
                        ps_o[:],
                        exp_sb[h][:, t * 512 + u * P: t * 512 + (u + 1) * P],
                        vw_pad[:, t * VW_STRIDE + gh * 65:
                               t * VW_STRIDE + (gh + 1) * 65],
                        start=(t == 0), stop=(t == NT - 1),
                    )
                r = sp.tile([P, 1], f32, name="recip", tag="recip")
                nc.vector.reciprocal(r[:], ps_o[:, 64:65])
                rq = sp.tile([P, 1], f32, name="rq", tag="rq")
                col = j * NU + u
                nc.vector.tensor_mul(rq[:], r[:], qm_sb[:, col:col + 1])
                nc.vector.tensor_scalar_mul(
                    out_t[:, h * 64:(h + 1) * 64], ps_o[:, 0:64], rq[:])

            seq = [(pair, j) for pair in range(4) for j in range(NJ)]
            prev = None  # (pair, j, exp_sb, out_tiles)
            for pair, j in seq:
                exp_sb = [ep.tile([P, NT * 512], bf16, name=f"exp{h}", tag=f"exp{h}")
                          for h in range(2)]
                out_tiles = [op.tile([P, P], f32, name="out_t", tag="out_t")
                             for _ in range(NU)]
                for g in range(8):
                    emit_scores_exp(pair, j, g, exp_sb)
                    if prev is not None:
                        ppair, pj, pexp, pouts = prev
                        u, h = g // 2, g % 2
                        emit_av_group(ppair, pj, u, h, pexp, pouts[u])
                        if h == 1:
                            nc.sync.dma_start(
                                out.ap()[pj * 512 + u * P: pj * 512 + (u + 1) * P,
                                         ppair * P:(ppair + 1) * P],
                                pouts[u][:])
                prev = (pair, j, exp_sb, out_tiles)

            ppair, pj, pexp, pouts = prev
            for g in range(8):
                u, h = g // 2, g % 2
                emit_av_group(ppair, pj, u, h, pexp, pouts[u])
                if h == 1:
                    nc.sync.dma_start(
                        out.ap()[pj * 512 + u * P: pj * 512 + (u + 1) * P,
                                 ppair * P:(ppair + 1) * P],
                        pouts[u][:])
            ps_av_cm.__exit__(None, None, None)
            ps_s_cm.__exit__(None, None, None)

    nc.compile()
    return nc


def _get_nc():
    if "nc" not in _cached:
        _cached["nc"] = _build()
    return _cached["nc"]


def kernel(q, k, v, Wq, Wk, Wv, v_mask, q_mask):
    import ml_dtypes
    from concourse.bass_utils import run_bass_kernel_spmd

    bf16 = ml_dtypes.bfloat16
    nc = _get_nc()

    Wq_b = np.ascontiguousarray(Wq, dtype=bf16)
    Wk_b = np.ascontiguousarray(Wk, dtype=bf16)
    Wv_b = np.ascontiguousarray(Wv, dtype=bf16)

    in_maps = []
    for i in range(N_CORES):
        in_maps.append({
            "qT": np.ascontiguousarray(q[i].T, dtype=bf16),
            "kT": np.ascontiguousarray(k[i].T, dtype=bf16),
            "vT": np.ascontiguousarray(v[i].T, dtype=bf16),
            "Wq": Wq_b, "Wk": Wk_b, "Wv": Wv_b,
            "vm": np.ascontiguousarray(
                v_mask[i, :, 0].reshape(NT, P).T, dtype=np.float32),
            "qm": np.ascontiguousarray(
                q_mask[i, :, 0].reshape(NT, P).T, dtype=np.float32),
        })

    res = run_bass_kernel_spmd(nc, in_maps, core_ids=list(range(N_CORES)))
    _cached["last_result"] = res
    return np.stack([res.results[i]["out"] for i in range(N_CORES)], axis=0)


# revision 11
# speedup vs baseline: 185.7881x; 185.7881x over previous
"""Trainium2 multi-head attention kernel (8 NeuronCores, data-parallel over batch).

Reference computation (per batch element b):
    qw = q @ Wq, kw = k @ Wk, vw = v @ Wv   (each reshaped to [S, H=8, Dh=64])
    a  = softmax(qw kw^T / sqrt(Dh) - (1-v_mask)*1e10, axis=k)
    o  = (a @ vw) * q_mask

Per-core layout strategy (core i handles batch element i):
  - Host passes q^T/k^T/v^T ([D=512, S=2048], bf16) so the contraction dim (d)
    lands on SBUF partitions for the projection matmuls.
  - Projections produce qw^T/kw^T ([512, 2048]; head h on partitions
    (h%2)*64..+64 of tile h//2) and vw in natural layout, padded per head with
    a denominator-ones column and pre-scaled by v_mask ([128, 16*520]).
  - Scores are computed transposed ([sk, sq]) two heads at a time via PE
    row-tiling (K=64 at array rows 0-63 / 64-127), exp'd on ScalarE
    (scale=1/8 folded into the activation) into bf16.
  - AV uses exp^T as the stationary operand: out[sq,65] accumulates over sk
    in PSUM; col 64 is the softmax denominator (ones column of vw_pad).
  - Epilogue: out = psum[:, :64] * reciprocal(psum[:, 64]) * q_mask, DMA'd
    straight to DRAM in natural [S, 512] layout.
  - ScalarE (exp) is the bottleneck engine (~33.5M exps/core at 1 elem/
    lane/cycle @ 1.2 GHz). The projection matmuls are therefore interleaved
    into the first attention blocks' emission so exp starts as soon as
    qw^T/kw^T tile 0 exists, instead of after the whole projection phase.

softmax is computed without max subtraction: logits are ~N(0,1) for these
input distributions (|logit| < ~7), so exp() cannot overflow, and masked
keys are handled by zeroing vw_pad rows (identical to the reference's
additive -1e10 mask).
"""

import numpy as np

B, S, D = 8, 2048, 512
H, DH = 8, 64
P = 128            # partitions
NDC = D // P       # 4 d-chunks
NT = S // P        # 16 sk tiles
NJ = 4             # sq chunks of 512
NU = 4             # sq subtiles of 128 per chunk
VW_STRIDE = 520    # 8 heads * 65 (64 vw cols + 1 ones col)
N_CORES = 8

_cached = {}


def _build(repeats=1, no_av=False, exp_frac=1.0, av_frac=1.0):
    import concourse.bacc as bacc
    import concourse.mybir as mybir
    import concourse.tile as tile

    f32 = mybir.dt.float32
    bf16 = mybir.dt.bfloat16
    Exp = mybir.ActivationFunctionType.Exp

    nc = bacc.Bacc("TRN2", target_bir_lowering=False, debug=False,
                   num_devices=N_CORES)

    qT = nc.dram_tensor("qT", [D, S], bf16, kind="ExternalInput")
    kT = nc.dram_tensor("kT", [D, S], bf16, kind="ExternalInput")
    vT = nc.dram_tensor("vT", [D, S], bf16, kind="ExternalInput")
    Wq = nc.dram_tensor("Wq", [D, D], bf16, kind="ExternalInput")
    Wk = nc.dram_tensor("Wk", [D, D], bf16, kind="ExternalInput")
    Wv = nc.dram_tensor("Wv", [D, D], bf16, kind="ExternalInput")
    vm = nc.dram_tensor("vm", [P, NT], f32, kind="ExternalInput")
    qm = nc.dram_tensor("qm", [P, NT], f32, kind="ExternalInput")
    out = nc.dram_tensor("out", [S, D], f32, kind="ExternalOutput")

    with tile.TileContext(nc) as tc:
        with (
            tc.tile_pool(name="persist", bufs=1) as pp,
            tc.tile_pool(name="inpool", bufs=1) as ip,
            tc.tile_pool(name="exppool", bufs=2) as ep,
            tc.tile_pool(name="outstage", bufs=8) as op,
            tc.tile_pool(name="small", bufs=8) as sp,
            tc.tile_pool(name="ps_proj", bufs=2, space="PSUM") as ps_p,
            tc.tile_pool(name="ps_scores", bufs=2, space="PSUM") as ps_s,
            tc.tile_pool(name="ps_av", bufs=2, space="PSUM") as ps_av,
        ):
            # persistent activations
            qwT = [pp.tile([P, S], bf16, name=f"qwT{i}", tag=f"qwT{i}")
                   for i in range(NDC)]
            kwT = [pp.tile([P, S], bf16, name=f"kwT{i}", tag=f"kwT{i}")
                   for i in range(NDC)]
            vw_pad = pp.tile([P, NT * VW_STRIDE], bf16, name="vw_pad",
                             tag="vw_pad")
            vm_sb = pp.tile([P, NT], f32, name="vm_sb", tag="vm")
            qm_sb = pp.tile([P, NT], f32, name="qm_sb", tag="qm")
            ones8 = pp.tile([P, H], f32, name="ones8", tag="ones8")
            scratch8 = pp.tile([P, H], f32, name="scratch8", tag="scratch8")

            qT_sb = [ip.tile([P, S], bf16, name=f"qT{i}", tag=f"qT{i}")
                     for i in range(NDC)]
            kT_sb = [ip.tile([P, S], bf16, name=f"kT{i}", tag=f"kT{i}")
                     for i in range(NDC)]
            vT_sb = [ip.tile([P, S], bf16, name=f"vT{i}", tag=f"vT{i}")
                     for i in range(NDC)]
            Wq_sb = [ip.tile([P, D], bf16, name=f"Wq{i}", tag=f"Wq{i}")
                     for i in range(NDC)]
            Wk_sb = [ip.tile([P, D], bf16, name=f"Wk{i}", tag=f"Wk{i}")
                     for i in range(NDC)]
            Wv_sb = [ip.tile([P, D], bf16, name=f"Wv{i}", tag=f"Wv{i}")
                     for i in range(NDC)]

            def rep_body(rep):
                # ---- input DMAs (q/k first: the earliest-needed tiles) ----
                nc.sync.dma_start(vm_sb[:], vm.ap())
                nc.sync.dma_start(qm_sb[:], qm.ap())
                nc.gpsimd.memset(ones8[:], 1.0)
                # warm the ACT exp table while DMAs run
                nc.scalar.activation(scratch8[:], ones8[:], Exp)
                for dc in range(NDC):
                    sl = slice(dc * P, (dc + 1) * P)
                    nc.sync.dma_start(qT_sb[dc][:], qT.ap()[sl, :])
                    nc.sync.dma_start(kT_sb[dc][:], kT.ap()[sl, :])
                    nc.sync.dma_start(Wq_sb[dc][:], Wq.ap()[sl, :])
                    nc.sync.dma_start(Wk_sb[dc][:], Wk.ap()[sl, :])
                for dc in range(NDC):
                    sl = slice(dc * P, (dc + 1) * P)
                    nc.sync.dma_start(vT_sb[dc][:], vT.ap()[sl, :])
                    nc.sync.dma_start(Wv_sb[dc][:], Wv.ap()[sl, :])

                # ---- projection work, chopped into PSUM-group closures ----
                def qk_group(W_sb, xT_sb, dstT, i, n):
                    def emit():
                        ps = ps_p.tile([P, 512], f32, name="ps_proj",
                                       tag="ps_proj")
                        for dc in range(NDC):
                            nc.tensor.matmul(
                                ps[:],
                                W_sb[dc][:, i * P:(i + 1) * P],
                                xT_sb[dc][:, n * 512:(n + 1) * 512],
                                start=(dc == 0), stop=(dc == NDC - 1),
                            )
                        nc.vector.tensor_copy(
                            dstT[i][:, n * 512:(n + 1) * 512], ps[:])
                    return emit

                def vw_group(m):
                    def emit():
                        ps = ps_p.tile([P, 512], f32, name="ps_projv",
                                       tag="ps_proj")
                        for dc in range(NDC):
                            nc.tensor.matmul(
                                ps[:],
                                vT_sb[dc][:, m * P:(m + 1) * P],
                                Wv_sb[dc][:],
                                start=(dc == 0), stop=(dc == NDC - 1),
                            )
                        base = m * VW_STRIDE
                        lay = vw_pad[:, base:base + VW_STRIDE].rearrange(
                            "p (h c) -> p h c", c=65)
                        nc.vector.tensor_scalar_mul(
                            lay[:, :, 0:64],
                            ps.rearrange("p (h c) -> p h c", c=64),
                            vm_sb[:, m:m + 1])
                        nc.vector.tensor_scalar_mul(
                            lay[:, :, 64:65],
                            ones8.rearrange("p (h c) -> p h c", c=1),
                            vm_sb[:, m:m + 1])
                    return emit

                # o-tile 0 of qw^T/kw^T up front: pair-0 scores need it
                for n in range(NJ):
                    qk_group(Wq_sb, qT_sb, qwT, 0, n)()
                    qk_group(Wk_sb, kT_sb, kwT, 0, n)()

                # remaining proj groups, drained 2-per-g-slot during the
                # first attention blocks (vw first: AV of block 0 needs it)
                proj_queue = [vw_group(m) for m in range(NT)]
                for i in range(1, NDC):
                    for n in range(NJ):
                        proj_queue.append(qk_group(Wq_sb, qT_sb, qwT, i, n))
                        proj_queue.append(qk_group(Wk_sb, kT_sb, kwT, i, n))

                # ---- attention ----
                def emit_scores_exp(pair, j, g, exp_sb):
                    for h in range(2):
                        prt = slice(h * 64, (h + 1) * 64)
                        ps = ps_s.tile([P, 1024], f32, name="ps_scores",
                                       tag="scores")
                        for tt in range(2):
                            t = 2 * g + tt
                            nc.tensor.matmul(
                                ps[:, tt * 512:(tt + 1) * 512],
                                kwT[pair][prt, t * P:(t + 1) * P],
                                qwT[pair][prt, j * 512:(j + 1) * 512],
                                start=True, stop=True,
                            )
                        ew = int(1024 * exp_frac)
                        nc.scalar.activation(
                            exp_sb[h][:, (2 * g) * 512:(2 * g) * 512 + ew],
                            ps[:, 0:ew], Exp, scale=0.125)

                def emit_av_group(pair, j, u, h, exp_sb, out_t):
                    gh = pair * 2 + h   # global head for vw_pad cols
                    ps_o = ps_av.tile([P, 65], f32, name="ps_o", tag="av")
                    for t in range(int(NT * av_frac)):
                        nc.tensor.matmul(
                            ps_o[:],
                            exp_sb[h][:, t * 512 + u * P:
                                      t * 512 + (u + 1) * P],
                            vw_pad[:, t * VW_STRIDE + gh * 65:
                                   t * VW_STRIDE + (gh + 1) * 65],
                            start=(t == 0), stop=(t == int(NT * av_frac) - 1),
                        )
                    r = sp.tile([P, 1], f32, name="recip", tag="recip")
                    nc.vector.reciprocal(r[:], ps_o[:, 64:65])
                    rq = sp.tile([P, 1], f32, name="rq", tag="rq")
                    col = j * NU + u
                    nc.vector.tensor_mul(rq[:], r[:], qm_sb[:, col:col + 1])
                    nc.vector.tensor_scalar_mul(
                        out_t[:, h * 64:(h + 1) * 64], ps_o[:, 0:64], rq[:])

                def emit_out_dma(ppair, pj, u, out_t):
                    nc.sync.dma_start(
                        out.ap()[pj * 512 + u * P: pj * 512 + (u + 1) * P,
                                 ppair * P:(ppair + 1) * P],
                        out_t[:])

                seq = [(pair, j) for pair in range(4) for j in range(NJ)]
                prev = None
                for pair, j in seq:
                    exp_sb = [ep.tile([P, NT * 512], bf16,
                                      name=f"exp{h}", tag=f"exp{h}")
                              for h in range(2)]
                    out_tiles = [op.tile([P, P], f32, name="out_t",
                                         tag="out_t") for _ in range(NU)]
                    for g in range(8):
                        emit_scores_exp(pair, j, g, exp_sb)
                        for _ in range(2):
                            if proj_queue:
                                proj_queue.pop(0)()
                        if prev is not None and not no_av:
                            ppair, pj, pexp, pouts = prev
                            u, h = g // 2, g % 2
                            emit_av_group(ppair, pj, u, h, pexp, pouts[u])
                            if h == 1:
                                emit_out_dma(ppair, pj, u, pouts[u])
                    prev = (pair, j, exp_sb, out_tiles)

                if not no_av:
                    ppair, pj, pexp, pouts = prev
                    for g in range(8):
                        u, h = g // 2, g % 2
                        emit_av_group(ppair, pj, u, h, pexp, pouts[u])
                        if h == 1:
                            emit_out_dma(ppair, pj, u, pouts[u])

            for rep in range(repeats):
                rep_body(rep)

    nc.compile()
    return nc


def _get_nc(repeats=1, no_av=False, exp_frac=1.0, av_frac=1.0):
    key = f"nc{repeats}_{no_av}_{exp_frac}_{av_frac}"
    if key not in _cached:
        _cached[key] = _build(repeats, no_av=no_av, exp_frac=exp_frac,
                              av_frac=av_frac)
    return _cached[key]


def make_in_maps(q, k, v, Wq, Wk, Wv, v_mask, q_mask):
    import ml_dtypes
    bf16 = ml_dtypes.bfloat16
    Wq_b = np.ascontiguousarray(Wq, dtype=bf16)
    Wk_b = np.ascontiguousarray(Wk, dtype=bf16)
    Wv_b = np.ascontiguousarray(Wv, dtype=bf16)
    in_maps = []
    for i in range(N_CORES):
        in_maps.append({
            "qT": np.ascontiguousarray(q[i].T, dtype=bf16),
            "kT": np.ascontiguousarray(k[i].T, dtype=bf16),
            "vT": np.ascontiguousarray(v[i].T, dtype=bf16),
            "Wq": Wq_b, "Wk": Wk_b, "Wv": Wv_b,
            "vm": np.ascontiguousarray(
                v_mask[i, :, 0].reshape(NT, P).T, dtype=np.float32),
            "qm": np.ascontiguousarray(
                q_mask[i, :, 0].reshape(NT, P).T, dtype=np.float32),
        })
    return in_maps


def _run_once(in_maps):
    from concourse.bass_utils import run_bass_kernel_spmd

    nc = _get_nc()
    res = run_bass_kernel_spmd(nc, in_maps, core_ids=list(range(N_CORES)))
    out = np.stack([res.results[i]["out"] for i in range(N_CORES)], axis=0)
    if not np.isfinite(out).all():
        raise RuntimeError("kernel produced non-finite values")
    if not np.abs(out).sum():
        raise RuntimeError("kernel produced all-zero output")
    return out


def _run_in_subprocess(q, k, v, Wq, Wk, Wv, v_mask, q_mask):
    """Last-resort retry in a fresh process (fresh device client), for
    transient NRT/relay failures that poison the in-process jax client."""
    import os
    import subprocess
    import sys
    import tempfile

    with tempfile.TemporaryDirectory() as td:
        np.savez(os.path.join(td, "in.npz"), q=q, k=k, v=v, Wq=Wq, Wk=Wk,
                 Wv=Wv, v_mask=v_mask, q_mask=q_mask)
        code = (
            "import sys, numpy as np\n"
            f"sys.path.insert(0, {os.path.dirname(os.path.abspath(__file__))!r})\n"
            "import kernel\n"
            f"d = np.load({os.path.join(td, 'in.npz')!r})\n"
            "out = kernel.kernel(**{k: d[k] for k in d.files})\n"
            f"np.save({os.path.join(td, 'out.npy')!r}, out)\n"
        )
        env = dict(os.environ)
        env["BASS_KERNEL_NO_SUBPROC"] = "1"
        subprocess.run([sys.executable, "-c", code], check=True, env=env,
                       timeout=1200)
        return np.load(os.path.join(td, "out.npy"))


def kernel(q, k, v, Wq, Wk, Wv, v_mask, q_mask):
    import os

    q = np.asarray(q, dtype=np.float32)
    k = np.asarray(k, dtype=np.float32)
    v = np.asarray(v, dtype=np.float32)
    in_maps = make_in_maps(q, k, v, Wq, Wk, Wv, v_mask, q_mask)

    last_err = None
    for _attempt in range(3):
        try:
            return _run_once(in_maps)
        except Exception as e:  # transient device errors: retry
            last_err = e
            import time as _time
            _time.sleep(2.0)
    if os.environ.get("BASS_KERNEL_NO_SUBPROC") != "1":
        try:
            return _run_in_subprocess(q, k, v, Wq, Wk, Wv, v_mask, q_mask)
        except Exception:
            pass
    raise last_err
